# revision 60
# baseline (speedup 1.0000x reference)
"""Trainium2 Bass kernel for nn_CrossAttention (B=4, C=256, H=W=64).

Sharding: 8 cores = (batch b, branch br). Each core computes ONE branch's
full [1, N] output row for its batch:
  br=0: q,k from x1, v from x2;  br=1: q,k from x2, v from x1.
Host passes role-named inputs (xqk, xv, xcr=x1-for-combine) so the SPMD
program is branch-agnostic. This halves the k/v projection work vs
query-half sharding (no duplication across the batch pair).

Per core, for all N=4096 query rows i:
  q = Wq xqk + bq        [32, 4096] stored 4x row-replicated as q4 [128, N]
  k = Wk xqk             [32, 4096] 4x row-replicated   (bk softmax-invariant)
  vT = (Wv xv)^T         [4096, 256] bf16  (bv folded into bc_eff on host)
  S^T[j, i] = k_j . q_i  ; E = exp(S^T) bf16  (|S| <~ 30, exp safe in f32)
  r[i] = sum_j E[j, i]   via DVE bf16 accumulation (2 interleaved accums)
                         + one K=128 ones-matmul fold per block
  1/r via reciprocal_approx_fast; broadcast to 128 partitions via a K=1
  ones outer-product matmul (no DRAM round trip)
  att = (vT^T E) / r ; comb = Wc [xcr; att] + bc_eff ; out = sum_c |comb|

Pipelining: 8 blocks of 512 query cols; within a block the attended
matmuls run one j-group behind the score matmuls (PE never waits on Act
exp); each block's tail (r fold/recip/broadcast/normalize/combine) is
issued interleaved into the NEXT block's matmul stream, so the PE queue
never drains and HAM stays at 2.4 GHz. The xcr combine input is DMA'd
at phase-2 start, hidden under the first block's compute.
PSUM: 4 score staging + 3 attended accumulators + 1 rotating tail bank.
"""

import numpy as np
import ml_dtypes

import concourse.bass as bass
import concourse.bacc as bacc
import concourse.tile as tile
import concourse.mybir as mybir
from concourse.bass_utils import run_bass_kernel_spmd

B, C, HH, WW = 4, 256, 64, 64
N = HH * WW          # 4096
CQK = 32
IH = N // 2
NCORES = 8
NJC = N // 128       # 32 key-dim 128-chunks
NG = NJC // 2        # 16 groups of 2 key-chunks
NBLK = N // 512      # 8 query blocks

F32 = mybir.dt.float32
F32R = mybir.dt.float32r
BF16 = mybir.dt.bfloat16
F16 = mybir.dt.float16
AF = mybir.ActivationFunctionType


def build_program(nc, tc):
    # ---- DRAM I/O ------------------------------------------------------
    dram = {}
    # xqk/wq/wk in fp16: x quantization at 0.05% keeps the score logits
    # accurate (bf16's 0.4% would shift competing logits too much), the
    # projections accumulate in f32, and fp16 matmuls run at full rate.
    for name, shape, dt in [
        ("xqk", [2, 128, N], F16), ("xv", [2, 128, N], BF16),
        ("xcr", [2, 128, N], BF16),
        ("wqk", [128, 2, 256], F16),       # wq || wk along last axis
        ("wvca", [128, 2, 3 * C], BF16),   # wv || wcx || wca
        ("bqe", [128, 3], F32),            # bq || bce
    ]:
        dram[name] = nc.dram_tensor(name, shape, dt, kind="ExternalInput").ap()
    out_d = nc.dram_tensor("out", [1, N], F32, kind="ExternalOutput").ap()

    import contextlib
    with contextlib.ExitStack() as ctx:
        persist = ctx.enter_context(tc.tile_pool(name="persist", bufs=1))

        wqk_sb = persist.tile([128, 2, 256], F16, tag="wqk")
        wvca_sb = persist.tile([128, 2, 3 * C], BF16, tag="wvca")
        bqe_sb = persist.tile([128, 3], F32, tag="bqe")
        ones_bf = persist.tile([128, 1], BF16, tag="ones")
        ones_row = persist.tile([1, 128], BF16, tag="ones_row")

        # weights packed host-side into 3 tensors = 3 dma_starts (each
        # start costs ~630ns of sequencer issue time), issued from the
        # (otherwise idle) scalar queue so they don't delay the x loads.
        nc.scalar.dma_start(out=wqk_sb, in_=dram["wqk"])
        nc.scalar.dma_start(out=wvca_sb, in_=dram["wvca"])
        nc.scalar.dma_start(out=bqe_sb, in_=dram["bqe"])
        nc.vector.memset(ones_bf, 1.0)
        nc.vector.memset(ones_row, 1.0)

        def wq_s(kc):
            return wqk_sb[:, kc, 0:128]

        def wk_s(kc):
            return wqk_sb[:, kc, 128:256]

        def wv_s(kc):
            return wvca_sb[:, kc, 0:C]

        def wcx_s(kc, c2):
            return wvca_sb[:, kc, bass.ds(C + c2 * 128, 128)]

        def wca_s(kc, c2):
            return wvca_sb[:, kc, bass.ds(2 * C + c2 * 128, 128)]

        bq_sb = bqe_sb[:, 0:1]
        bce_sb = bqe_sb[:, 1:3]

        q4_sb = persist.tile([128, N], F32R, tag="q4")
        k4_sb = [persist.tile([128, IH], F32R, tag=f"k{h}", name=f"k{h}")
                 for h in range(2)]
        vT_sb = [persist.tile([128, (NJC // 2) * C], BF16, tag=f"vt{h}",
                              name=f"vt{h}") for h in range(2)]
        att_sb = [persist.tile([128, N], BF16, tag=f"att{c2}",
                               name=f"att{c2}") for c2 in range(2)]
        xcr_sb = persist.tile([128, 2, N], BF16, tag="xcr")

        # ---- phase 1: projections -------------------------------------
        with tc.tile_pool(name="proj_sb", bufs=4) as proj_sb, \
             tc.tile_pool(name="ps_kq", bufs=3, space="PSUM") as ps_kq, \
             tc.tile_pool(name="ps_vt", bufs=2, space="PSUM") as ps_vt:

            xq = [proj_sb.tile([128, 2, IH], F16, tag="xq", name=f"xq{h}")
                  for h in range(2)]
            xvt = [proj_sb.tile([128, 2, IH], BF16, tag="xv", name=f"xv{h}")
                   for h in range(2)]

            def load_half(src, dst, h, eng, nchunk):
                # chunks in consumption order: dma_start issue costs
                # ~630ns (sync) / ~1us (gpsimd SWDGE) of sequencer time,
                # so chunk count balances issue rate against letting the
                # first projection start early. xqk via sync, xv via the
                # idle gpsimd queue so both streams issue in parallel.
                cw = IH // nchunk
                for jb in range(nchunk):
                    sl = bass.ds(jb * cw, cw)
                    for kc in range(2):
                        eng.dma_start(
                            out=dst[:, kc, sl],
                            in_=dram[src][kc][:, h * IH + jb * cw:
                                              h * IH + (jb + 1) * cw])

            load_half("xqk", xq[0], 0, nc.sync, 4)
            load_half("xv", xvt[0], 0, nc.gpsimd, 2)
            load_half("xqk", xq[1], 1, nc.sync, 2)
            load_half("xv", xvt[1], 1, nc.scalar, 2)

            def qk_proj(h):
                xap = [xq[h][:, 0, :], xq[h][:, 1, :]]
                for jb in range(4):
                    sl = bass.ts(jb, 512)
                    osl = bass.ts(h * 4 + jb, 512)
                    qp = ps_kq.tile([128, 512], F32, tag="kq", name="qp")
                    for kc in range(2):
                        nc.tensor.matmul(qp, wq_s(kc), xap[kc][:, sl],
                                         start=(kc == 0), stop=(kc == 1))
                    nc.scalar.activation(q4_sb[:, osl], qp, AF.Identity,
                                         bias=bq_sb)
                    kp = ps_kq.tile([128, 512], F32, tag="kq", name="kp")
                    for kc in range(2):
                        nc.tensor.matmul(kp, wk_s(kc), xap[kc][:, sl],
                                         start=(kc == 0), stop=(kc == 1))
                    # k drain on DVE: keeps Act free for the first block's
                    # exps (Act otherwise enters phase 2 with a backlog)
                    nc.vector.tensor_copy(k4_sb[h][:, sl], kp)

            def v_proj(h):
                xap = [xvt[h][:, 0, :], xvt[h][:, 1, :]]
                for g in range(4):
                    vtp = ps_vt.tile([128, 4, C], F32, tag="vt", name="vtp")
                    for s in range(4):
                        jsub = g * 4 + s
                        for kc in range(2):
                            nc.tensor.matmul(
                                vtp[:, s, :],
                                xap[kc][:, bass.ts(jsub, 128)],
                                wv_s(kc),
                                start=(kc == 0), stop=(kc == 1))
                    nc.vector.tensor_copy(
                        vT_sb[h][:, bass.ds(g * 4 * C, 4 * C)],
                        vtp.rearrange("p a c -> p (a c)"))

            qk_proj(0)
            qk_proj(1)
            # combine input load: issued on the scalar queue HERE so its
            # transfer starts only after the projection inputs are on the
            # wire (needed first by block 0's tail, ~20us into phase 2).
            for kc in range(2):
                nc.scalar.dma_start(out=xcr_sb[:, kc, :], in_=dram["xcr"][kc])
            v_proj(0)
            v_proj(1)

        # ---- phase 2: attention + fused combine, 1-block pipelined ----
        with tc.tile_pool(name="attn_sb", bufs=1) as attn_sb, \
             tc.tile_pool(name="ps_st", bufs=1, space="PSUM") as ps_st, \
             tc.tile_pool(name="ps_att", bufs=1, space="PSUM") as ps_att, \
             tc.tile_pool(name="ps_tail", bufs=1, space="PSUM") as ps_tail:

            state = {}

            def tail_ops(p, g, pst, epi=False):
                pisl = bass.ts(p, 512)
                if g == 0:
                    for c2 in range(2):
                        nc.vector.tensor_copy(att_sb[c2][:, pisl],
                                              pst["attp"][c2])
                elif g == 1:
                    rft = ps_tail.tile([1, 512], F32, tag="tail", name="rft")
                    nc.tensor.matmul(rft, ones_bf, pst["racc"][0],
                                     start=True, stop=False)
                    nc.tensor.matmul(rft, ones_bf, pst["racc"][1],
                                     start=False, stop=True)
                    pst["rft"] = rft
                elif g == 2:
                    rr = attn_sb.tile([1, 512], F32, tag="rr", bufs=2,
                                      name="rr")
                    nc.vector.reciprocal_approx_fast(out=rr, in_=pst["rft"])
                    rr_bf = attn_sb.tile([1, 512], BF16, tag="rr_bf", bufs=2,
                                         name="rr_bf")
                    nc.vector.tensor_copy(rr_bf, rr)
                    pst["rr_bf"] = rr_bf
                elif g == 3:
                    rrb = ps_tail.tile([128, 512], F32, tag="tail",
                                       name="rrb")
                    nc.tensor.matmul(rrb, ones_row, pst["rr_bf"],
                                     start=True, stop=True)
                    pst["rrb"] = rrb
                elif g == 4:
                    for c2 in range(2):
                        a = att_sb[c2][:, pisl]
                        nc.vector.tensor_mul(a, a, pst["rrb"])
                elif g == 5 or g == 6:
                    c2 = 0 if g == 5 else 1
                    if epi:
                        # final block: scores are done, so the stp banks
                        # are free — dodging the tail-bank serialization
                        # shortens the end-of-kernel drain chain.
                        if c2 == 0:
                            pst["cpe"] = ps_st.tile([128, 4, 512], F32,
                                                    tag="stp", bufs=1,
                                                    name="cpe")
                        cp = pst["cpe"][:, c2, :]
                    else:
                        cp = ps_tail.tile([128, 512], F32, tag="tail",
                                          name=f"cp{c2}")
                    for kc in range(2):
                        nc.tensor.matmul(cp, wcx_s(kc, c2),
                                         xcr_sb[:, kc, pisl],
                                         start=(kc == 0), stop=False)
                    for kc in range(2):
                        nc.tensor.matmul(cp, wca_s(kc, c2),
                                         att_sb[kc][:, pisl],
                                         start=False, stop=(kc == 1))
                    absb = attn_sb.tile([128, 512], BF16, tag="absb",
                                        bufs=4, name=f"absb{c2}")
                    nc.scalar.activation(absb, cp, AF.Abs,
                                         bias=bce_sb[:, c2:c2 + 1])
                    pst[f"absb{c2}"] = absb
                elif g == 7:
                    outp = ps_tail.tile([1, 512], F32, tag="tail",
                                        name="outp")
                    nc.tensor.matmul(outp, ones_bf, pst["absb0"],
                                     start=True, stop=False)
                    nc.tensor.matmul(outp, ones_bf, pst["absb1"],
                                     start=False, stop=True)
                    pst["outp"] = outp
                    osb = attn_sb.tile([1, 512], F32, tag="osb", bufs=2,
                                       name="osb")
                    nc.vector.tensor_copy(osb, outp)
                    nc.sync.dma_start(out=out_d[0:1, pisl], in_=osb)

            def attended(st, g, first, last):
                jh = g // (NG // 2)
                est = st["est"][(g // 2) % 4]
                for t in range(2):
                    jloc = (2 * g + t) - jh * (NJC // 2)
                    for c2 in range(2):
                        nc.tensor.matmul(
                            st["attp"][c2],
                            vT_sb[jh][:, bass.ds(jloc * C + c2 * 128, 128)],
                            est[:, 2 * (g % 2) + t, :],
                            start=(first and t == 0),
                            stop=(last and t == 1))

            for mi in range(NBLK + 1):
                cur = mi if mi < NBLK else None
                if cur is not None:
                    isl = bass.ts(cur, 512)
                    st = {
                        "attp": [ps_att.tile([128, 512], F32, tag="attp",
                                             bufs=3, name=f"attp{c2}")
                                 for c2 in range(2)],
                        "racc": [attn_sb.tile([128, 512], BF16, tag="racc",
                                              bufs=4, name=f"racc{t}")
                                 for t in range(2)],
                        "est": [None] * 4,
                    }
                    state[mi] = st
                for g in range(16 if cur is not None else 16):
                    if cur is not None:
                        if g % 2 == 0:
                            # score QUAD: 4 chunks (2 groups) issued as
                            # back-to-back matmuls on disjoint 32-row PE
                            # strips — they stream concurrently, so the
                            # quad costs ~1 matmul, not 4. The 4x q/k row
                            # replication provides all 4 strips. One
                            # [128,4,512] exp covers the whole quad
                            # (amortizes Act per-instruction overhead).
                            stp = ps_st.tile([128, 4, 512], F32,
                                             tag="stp", bufs=1, name="stp")
                            jh = g // (NG // 2)
                            for u in range(4):
                                c = 4 * (g // 2) + u
                                jloc = c - jh * (NJC // 2)
                                nc.tensor.matmul(
                                    stp[:, u, :],
                                    k4_sb[jh][bass.ds(32 * u, 32),
                                              bass.ts(jloc, 128)],
                                    q4_sb[bass.ds(32 * u, 32), isl],
                                    start=True, stop=True,
                                    tile_position=(32 * u, 0))
                            est = attn_sb.tile([128, 4, 512], BF16,
                                               tag="est", bufs=4,
                                               name="est")
                            st["est"][(g // 2) % 4] = est
                            nc.scalar.activation(
                                est.rearrange("p a n -> p (a n)"),
                                stp.rearrange("p a n -> p (a n)"), AF.Exp)
                        est = st["est"][(g // 2) % 4]
                        # attended runs TWO groups behind the scores so
                        # the sem->exp->sem latency chain never paces the
                        # PE stream.
                        if g > 1:
                            attended(st, g - 2, first=(g == 2), last=False)
                        # DVE r-accumulation (bf16, 2 interleaved accums)
                        for t in range(2):
                            sub = 2 * (g % 2) + t
                            if g == 0:
                                nc.vector.tensor_copy(st["racc"][t],
                                                      est[:, sub, :])
                            else:
                                nc.vector.tensor_add(st["racc"][t],
                                                     st["racc"][t],
                                                     est[:, sub, :])
                    if mi > 0 and g % 2 == 0:
                        tail_ops(mi - 1, g // 2, state[mi - 1],
                                 epi=(cur is None))
                if cur is not None:
                    attended(st, 14, first=False, last=False)
                    attended(st, 15, first=False, last=True)


_NC_CACHE = {}


def _get_nc():
    if "nc" not in _NC_CACHE:
        nc = bacc.Bacc("TRN2", debug=False, enable_asserts=False,
                       target_bir_lowering=False, enable_partition_id=False)
        with tile.TileContext(nc) as tc:
            build_program(nc, tc)
        nc.compile()
        _NC_CACHE["nc"] = nc
    return _NC_CACHE["nc"]


def host_inputs(x1, x2, Wq, bq, Wk, bk, Wv, bv, Wc, bc):
    """Build the 8 per-core input maps (host-side sharding/layout only)."""
    f = np.float32
    x1 = np.asarray(x1, f); x2 = np.asarray(x2, f)
    Wq = np.asarray(Wq, f); bq = np.asarray(bq, f)
    Wk = np.asarray(Wk, f)
    Wv = np.asarray(Wv, f); bv = np.asarray(bv, f)
    Wc = np.asarray(Wc, f); bc = np.asarray(bc, f)

    # 4x row-replicated q/k projection weights, packed [128, kc, out]
    Wq4 = np.tile(Wq, (4, 1))            # [128, 256]
    Wk4 = np.tile(Wk, (4, 1))
    wq_t = Wq4.T.reshape(2, 128, 128).transpose(1, 0, 2)
    wk_t = Wk4.T.reshape(2, 128, 128).transpose(1, 0, 2)
    wqk = np.ascontiguousarray(
        np.concatenate([wq_t, wk_t], axis=2)).astype(np.float16)
    wv_t = Wv.T.reshape(2, 128, C).transpose(1, 0, 2)
    WcT = np.ascontiguousarray(Wc.T)     # [512, 256]
    wcx_t = WcT[:C].reshape(2, 128, C).transpose(1, 0, 2)
    wca_t = WcT[C:].reshape(2, 128, C).transpose(1, 0, 2)
    wvca = np.ascontiguousarray(np.concatenate(
        [wv_t, wcx_t, wca_t], axis=2)).astype(ml_dtypes.bfloat16)
    bq4 = np.tile(bq, 4)                                    # [128]
    bce = (bc + Wc[:, C:] @ bv).reshape(2, 128).T           # [128, 2]
    bqe = np.ascontiguousarray(
        np.concatenate([bq4[:, None], bce], axis=1), np.float32)

    in_maps = []
    bf = ml_dtypes.bfloat16
    for core in range(NCORES):
        b, br = divmod(core, 2)
        x1f = np.ascontiguousarray(x1[b].reshape(C, N).reshape(2, 128, N))
        x2f = np.ascontiguousarray(x2[b].reshape(C, N).reshape(2, 128, N))
        in_maps.append({
            "xqk": (x1f if br == 0 else x2f).astype(np.float16),
            "xv": (x2f if br == 0 else x1f).astype(bf),
            "xcr": x1f.astype(bf),
            "wqk": wqk, "wvca": wvca, "bqe": bqe,
        })
    return in_maps


def assemble(results):
    """results: list of 8 dicts with 'out' [1, N] -> (out1, out2) full."""
    outs = []
    for br in range(2):
        full = np.empty((B, 1, HH, WW), np.float32)
        for b in range(B):
            full[b, 0] = results[2 * b + br]["out"][0].reshape(HH, WW)
        outs.append(full)
    return outs[0], outs[1]


def kernel(x1, x2, Wq, bq, Wk, bk, Wv, bv, Wc, bc):
    in_maps = host_inputs(x1, x2, Wq, bq, Wk, bk, Wv, bv, Wc, bc)
    nc = _get_nc()
    res = run_bass_kernel_spmd(nc, in_maps, core_ids=list(range(NCORES)))
    return assemble(res.results)


# revision 67
# speedup vs baseline: 1.0169x; 1.0169x over previous
"""Trainium2 Bass kernel for nn_CrossAttention (B=4, C=256, H=W=64).

Sharding: 8 cores = (batch b, branch br). Each core computes ONE branch's
full [1, N] output row for its batch:
  br=0: q,k from x1, v from x2;  br=1: q,k from x2, v from x1.
Host passes role-named inputs (xqk, xv, xcr=x1-for-combine) so the SPMD
program is branch-agnostic. This halves the k/v projection work vs
query-half sharding (no duplication across the batch pair).

Per core, for all N=4096 query rows i:
  q = Wq xqk + bq        [32, 4096] stored 4x row-replicated as q4 [128, N]
  k = Wk xqk             [32, 4096] 4x row-replicated   (bk softmax-invariant)
  vT = (Wv xv)^T         [4096, 256] bf16  (bv folded into bc_eff on host)
  S^T[j, i] = k_j . q_i  ; E = exp(S^T) bf16  (|S| <~ 30, exp safe in f32)
  r[i] = sum_j E[j, i]   via DVE bf16 accumulation (2 interleaved accums)
                         + one K=128 ones-matmul fold per block
  1/r via reciprocal_approx_fast; broadcast to 128 partitions via a K=1
  ones outer-product matmul (no DRAM round trip)
  att = (vT^T E) / r ; comb = Wc [xcr; att] + bc_eff ; out = sum_c |comb|

Pipelining: 8 blocks of 512 query cols; within a block the attended
matmuls run one j-group behind the score matmuls (PE never waits on Act
exp); each block's tail (r fold/recip/broadcast/normalize/combine) is
issued interleaved into the NEXT block's matmul stream, so the PE queue
never drains and HAM stays at 2.4 GHz. The xcr combine input is DMA'd
at phase-2 start, hidden under the first block's compute.
PSUM: 4 score staging + 3 attended accumulators + 1 rotating tail bank.
"""

import numpy as np
import ml_dtypes

import concourse.bass as bass
import concourse.bacc as bacc
import concourse.tile as tile
import concourse.mybir as mybir
from concourse.bass_utils import run_bass_kernel_spmd

B, C, HH, WW = 4, 256, 64, 64
N = HH * WW          # 4096
CQK = 32
IH = N // 2
NCORES = 8
NJC = N // 128       # 32 key-dim 128-chunks
NG = NJC // 2        # 16 groups of 2 key-chunks
NBLK = N // 512      # 8 query blocks

F32 = mybir.dt.float32
F32R = mybir.dt.float32r
BF16 = mybir.dt.bfloat16
F16 = mybir.dt.float16
AF = mybir.ActivationFunctionType


def build_program(nc, tc):
    # ---- DRAM I/O ------------------------------------------------------
    dram = {}
    # xqk/wq/wk in fp16: x quantization at 0.05% keeps the score logits
    # accurate (bf16's 0.4% would shift competing logits too much), the
    # projections accumulate in f32, and fp16 matmuls run at full rate.
    for name, shape, dt in [
        ("xqk", [2, 128, N], F16), ("xv", [2, 128, N], BF16),
        ("xcr", [2, 128, N], BF16),
        ("wqk", [128, 2, 256], F16),       # wq || wk along last axis
        ("wvca", [128, 2, 3 * C], BF16),   # wv || wcx || wca
        ("bqe", [128, 3], F32),            # bq || bce
    ]:
        dram[name] = nc.dram_tensor(name, shape, dt, kind="ExternalInput").ap()
    out_d = nc.dram_tensor("out", [1, N], F32, kind="ExternalOutput").ap()

    import contextlib
    with contextlib.ExitStack() as ctx:
        persist = ctx.enter_context(tc.tile_pool(name="persist", bufs=1))

        wqk_sb = persist.tile([128, 2, 256], F16, tag="wqk")
        wvca_sb = persist.tile([128, 2, 3 * C], BF16, tag="wvca")
        bqe_sb = persist.tile([128, 3], F32, tag="bqe")
        ones_bf = persist.tile([128, 1], BF16, tag="ones")
        ones_row = persist.tile([1, 128], BF16, tag="ones_row")

        # weights packed host-side into 3 tensors = 3 dma_starts (each
        # start costs ~630ns of sequencer issue time), issued from the
        # (otherwise idle) scalar queue so they don't delay the x loads.
        nc.scalar.dma_start(out=wqk_sb, in_=dram["wqk"])
        nc.scalar.dma_start(out=wvca_sb, in_=dram["wvca"])
        nc.scalar.dma_start(out=bqe_sb, in_=dram["bqe"])
        nc.vector.memset(ones_bf, 1.0)
        nc.vector.memset(ones_row, 1.0)

        def wq_s(kc):
            return wqk_sb[:, kc, 0:128]

        def wk_s(kc):
            return wqk_sb[:, kc, 128:256]

        def wv_s(kc):
            return wvca_sb[:, kc, 0:C]

        def wcx_s(kc, c2):
            return wvca_sb[:, kc, bass.ds(C + c2 * 128, 128)]

        def wca_s(kc, c2):
            return wvca_sb[:, kc, bass.ds(2 * C + c2 * 128, 128)]

        bq_sb = bqe_sb[:, 0:1]
        bce_sb = bqe_sb[:, 1:3]

        q4_sb = persist.tile([128, N], F32R, tag="q4")
        k4_sb = [persist.tile([128, IH], F32R, tag=f"k{h}", name=f"k{h}")
                 for h in range(2)]
        vT_sb = [persist.tile([128, (NJC // 2) * C], BF16, tag=f"vt{h}",
                              name=f"vt{h}") for h in range(2)]
        att_sb = [persist.tile([128, N], BF16, tag=f"att{c2}",
                               name=f"att{c2}") for c2 in range(2)]
        xcr_sb = persist.tile([128, 2, N], BF16, tag="xcr")

        # ---- phase 1: projections -------------------------------------
        with tc.tile_pool(name="proj_sb", bufs=4) as proj_sb, \
             tc.tile_pool(name="ps_kq", bufs=3, space="PSUM") as ps_kq, \
             tc.tile_pool(name="ps_vt", bufs=2, space="PSUM") as ps_vt:

            xq = [proj_sb.tile([128, 2, IH], F16, tag="xq", name=f"xq{h}")
                  for h in range(2)]
            xvt = [proj_sb.tile([128, 2, IH], BF16, tag="xv", name=f"xv{h}")
                   for h in range(2)]

            def load_half(src, dst, h, eng, nchunk):
                # chunks in consumption order: dma_start issue costs
                # ~630ns (sync) / ~1us (gpsimd SWDGE) of sequencer time,
                # so chunk count balances issue rate against letting the
                # first projection start early. xqk via sync, xv via the
                # idle gpsimd queue so both streams issue in parallel.
                cw = IH // nchunk
                for jb in range(nchunk):
                    sl = bass.ds(jb * cw, cw)
                    for kc in range(2):
                        eng.dma_start(
                            out=dst[:, kc, sl],
                            in_=dram[src][kc][:, h * IH + jb * cw:
                                              h * IH + (jb + 1) * cw])

            load_half("xqk", xq[0], 0, nc.sync, 4)
            load_half("xv", xvt[0], 0, nc.gpsimd, 2)
            load_half("xqk", xq[1], 1, nc.sync, 2)
            load_half("xv", xvt[1], 1, nc.scalar, 2)

            def qk_proj(h):
                xap = [xq[h][:, 0, :], xq[h][:, 1, :]]
                for jb in range(4):
                    sl = bass.ts(jb, 512)
                    osl = bass.ts(h * 4 + jb, 512)
                    qp = ps_kq.tile([128, 512], F32, tag="kq", name="qp")
                    for kc in range(2):
                        nc.tensor.matmul(qp, wq_s(kc), xap[kc][:, sl],
                                         start=(kc == 0), stop=(kc == 1))
                    nc.scalar.activation(q4_sb[:, osl], qp, AF.Identity,
                                         bias=bq_sb)
                    kp = ps_kq.tile([128, 512], F32, tag="kq", name="kp")
                    for kc in range(2):
                        nc.tensor.matmul(kp, wk_s(kc), xap[kc][:, sl],
                                         start=(kc == 0), stop=(kc == 1))
                    # k drain on DVE: keeps Act free for the first block's
                    # exps (Act otherwise enters phase 2 with a backlog)
                    nc.vector.tensor_copy(k4_sb[h][:, sl], kp)

            def v_proj(h):
                xap = [xvt[h][:, 0, :], xvt[h][:, 1, :]]
                for g in range(4):
                    vtp = ps_vt.tile([128, 4, C], F32, tag="vt", name="vtp")
                    for s in range(4):
                        jsub = g * 4 + s
                        for kc in range(2):
                            nc.tensor.matmul(
                                vtp[:, s, :],
                                xap[kc][:, bass.ts(jsub, 128)],
                                wv_s(kc),
                                start=(kc == 0), stop=(kc == 1))
                    nc.vector.tensor_copy(
                        vT_sb[h][:, bass.ds(g * 4 * C, 4 * C)],
                        vtp.rearrange("p a c -> p (a c)"))

            qk_proj(0)
            qk_proj(1)
            # combine input load: issued on the scalar queue HERE so its
            # transfer starts only after the projection inputs are on the
            # wire (needed first by block 0's tail, ~20us into phase 2).
            for kc in range(2):
                nc.scalar.dma_start(out=xcr_sb[:, kc, :], in_=dram["xcr"][kc])
            v_proj(0)
            v_proj(1)

        # ---- phase 2: attention + fused combine, 1-block pipelined ----
        with tc.tile_pool(name="attn_sb", bufs=1) as attn_sb, \
             tc.tile_pool(name="ps_st", bufs=1, space="PSUM") as ps_st, \
             tc.tile_pool(name="ps_att", bufs=1, space="PSUM") as ps_att, \
             tc.tile_pool(name="ps_tail", bufs=1, space="PSUM") as ps_tail:

            state = {}

            def tail_ops(p, g, pst, epi=False):
                pisl = bass.ts(p, 512)
                if g == 0:
                    for c2 in range(2):
                        nc.vector.tensor_copy(att_sb[c2][:, pisl],
                                              pst["attp"][c2])
                elif g == 1:
                    rft = ps_tail.tile([1, 512], F32, tag="tail", name="rft")
                    nc.tensor.matmul(rft, ones_bf, pst["racc"][0],
                                     start=True, stop=False)
                    nc.tensor.matmul(rft, ones_bf, pst["racc"][1],
                                     start=False, stop=True)
                    pst["rft"] = rft
                elif g == 2:
                    rr = attn_sb.tile([1, 512], F32, tag="rr", bufs=2,
                                      name="rr")
                    nc.vector.reciprocal_approx_fast(out=rr, in_=pst["rft"])
                    rr_bf = attn_sb.tile([1, 512], BF16, tag="rr_bf", bufs=2,
                                         name="rr_bf")
                    nc.vector.tensor_copy(rr_bf, rr)
                    pst["rr_bf"] = rr_bf
                elif g == 5:
                    rrb = ps_tail.tile([128, 512], F32, tag="tail",
                                       name="rrb")
                    nc.tensor.matmul(rrb, ones_row, pst["rr_bf"],
                                     start=True, stop=True)
                    pst["rrb"] = rrb
                elif g == 6:
                    for c2 in range(2):
                        a = att_sb[c2][:, pisl]
                        nc.vector.tensor_mul(a, a, pst["rrb"])
                elif g == 8 or g == 10:
                    c2 = 0 if g == 8 else 1
                    if epi:
                        # final block: scores are done, so the stp banks
                        # are free — dodging the tail-bank serialization
                        # shortens the end-of-kernel drain chain.
                        cp = ps_st.tile([128, 2, 512], F32, tag="stp",
                                        bufs=2, name=f"cpe{c2}")[:, 0, :]
                    else:
                        cp = ps_tail.tile([128, 512], F32, tag="tail",
                                          name=f"cp{c2}")
                    for kc in range(2):
                        nc.tensor.matmul(cp, wcx_s(kc, c2),
                                         xcr_sb[:, kc, pisl],
                                         start=(kc == 0), stop=False)
                    for kc in range(2):
                        nc.tensor.matmul(cp, wca_s(kc, c2),
                                         att_sb[kc][:, pisl],
                                         start=False, stop=(kc == 1))
                    absb = attn_sb.tile([128, 512], BF16, tag="absb",
                                        bufs=4, name=f"absb{c2}")
                    nc.scalar.activation(absb, cp, AF.Abs,
                                         bias=bce_sb[:, c2:c2 + 1])
                    pst[f"absb{c2}"] = absb
                elif g == 12:
                    outp = ps_tail.tile([1, 512], F32, tag="tail",
                                        name="outp")
                    nc.tensor.matmul(outp, ones_bf, pst["absb0"],
                                     start=True, stop=False)
                    nc.tensor.matmul(outp, ones_bf, pst["absb1"],
                                     start=False, stop=True)
                    pst["outp"] = outp
                elif g == 13:
                    osb = attn_sb.tile([1, 512], F32, tag="osb", bufs=2,
                                       name="osb")
                    nc.vector.tensor_copy(osb, pst["outp"])
                    nc.sync.dma_start(out=out_d[0:1, pisl], in_=osb)

            def attended(st, g, first, last):
                jh = g // (NG // 2)
                for t in range(2):
                    jloc = (2 * g + t) - jh * (NJC // 2)
                    for c2 in range(2):
                        nc.tensor.matmul(
                            st["attp"][c2],
                            vT_sb[jh][:, bass.ds(jloc * C + c2 * 128, 128)],
                            st["est"][g % 8][:, t, :],
                            start=(first and t == 0),
                            stop=(last and t == 1))

            for mi in range(NBLK + 1):
                cur = mi if mi < NBLK else None
                if cur is not None:
                    isl = bass.ts(cur, 512)
                    st = {
                        "attp": [ps_att.tile([128, 512], F32, tag="attp",
                                             bufs=3, name=f"attp{c2}")
                                 for c2 in range(2)],
                        "racc": [attn_sb.tile([128, 512], BF16, tag="racc",
                                              bufs=4, name=f"racc{t}")
                                 for t in range(2)],
                        "est": [None] * 8,
                    }
                    state[mi] = st
                for g in range(16 if cur is not None else 14):
                    if cur is not None:
                        if g % 2 == 0:
                            # score QUAD: 4 chunks (2 groups) issued as
                            # back-to-back matmuls on disjoint 32-row PE
                            # strips — they stream concurrently, so the
                            # quad costs ~1 matmul, not 4. The 4x q/k row
                            # replication provides all 4 strips.
                            stps = [ps_st.tile([128, 2, 512], F32,
                                               tag="stp", bufs=2,
                                               name=f"stp{u}")
                                    for u in range(2)]
                            st["stp"] = stps
                            jh = g // (NG // 2)
                            for u in range(4):
                                c = 4 * (g // 2) + u
                                jloc = c - jh * (NJC // 2)
                                nc.tensor.matmul(
                                    stps[u // 2][:, u % 2, :],
                                    k4_sb[jh][bass.ds(32 * u, 32),
                                              bass.ts(jloc, 128)],
                                    q4_sb[bass.ds(32 * u, 32), isl],
                                    start=True, stop=True,
                                    tile_position=(32 * u, 0))
                        stp = st["stp"][g % 2]
                        est = attn_sb.tile([128, 2, 512], BF16,
                                           tag="est", bufs=8, name="est")
                        st["est"][g % 8] = est
                        nc.scalar.activation(
                            est.rearrange("p a n -> p (a n)"),
                            stp.rearrange("p a n -> p (a n)"), AF.Exp)
                        # attended runs TWO groups behind the scores so
                        # the sem->exp->sem latency chain never paces the
                        # PE stream.
                        if g > 1:
                            attended(st, g - 2, first=(g == 2), last=False)
                        # DVE r-accumulation (bf16, 2 interleaved accums)
                        for t in range(2):
                            if g == 0:
                                nc.vector.tensor_copy(st["racc"][t],
                                                      est[:, t, :])
                            else:
                                nc.vector.tensor_add(st["racc"][t],
                                                     st["racc"][t],
                                                     est[:, t, :])
                    if mi > 0:
                        tail_ops(mi - 1, g, state[mi - 1], epi=(cur is None))
                if cur is not None:
                    attended(st, 14, first=False, last=False)
                    attended(st, 15, first=False, last=True)


_NC_CACHE = {}


def _get_nc():
    if "nc" not in _NC_CACHE:
        nc = bacc.Bacc("TRN2", debug=False, enable_asserts=False,
                       target_bir_lowering=False, enable_partition_id=False)
        with tile.TileContext(nc) as tc:
            build_program(nc, tc)
        nc.compile()
        _NC_CACHE["nc"] = nc
    return _NC_CACHE["nc"]


def host_inputs(x1, x2, Wq, bq, Wk, bk, Wv, bv, Wc, bc):
    """Build the 8 per-core input maps (host-side sharding/layout only)."""
    f = np.float32
    x1 = np.asarray(x1, f); x2 = np.asarray(x2, f)
    Wq = np.asarray(Wq, f); bq = np.asarray(bq, f)
    Wk = np.asarray(Wk, f)
    Wv = np.asarray(Wv, f); bv = np.asarray(bv, f)
    Wc = np.asarray(Wc, f); bc = np.asarray(bc, f)

    # 4x row-replicated q/k projection weights, packed [128, kc, out]
    Wq4 = np.tile(Wq, (4, 1))            # [128, 256]
    Wk4 = np.tile(Wk, (4, 1))
    wq_t = Wq4.T.reshape(2, 128, 128).transpose(1, 0, 2)
    wk_t = Wk4.T.reshape(2, 128, 128).transpose(1, 0, 2)
    wqk = np.ascontiguousarray(
        np.concatenate([wq_t, wk_t], axis=2)).astype(np.float16)
    wv_t = Wv.T.reshape(2, 128, C).transpose(1, 0, 2)
    WcT = np.ascontiguousarray(Wc.T)     # [512, 256]
    wcx_t = WcT[:C].reshape(2, 128, C).transpose(1, 0, 2)
    wca_t = WcT[C:].reshape(2, 128, C).transpose(1, 0, 2)
    wvca = np.ascontiguousarray(np.concatenate(
        [wv_t, wcx_t, wca_t], axis=2)).astype(ml_dtypes.bfloat16)
    bq4 = np.tile(bq, 4)                                    # [128]
    bce = (bc + Wc[:, C:] @ bv).reshape(2, 128).T           # [128, 2]
    bqe = np.ascontiguousarray(
        np.concatenate([bq4[:, None], bce], axis=1), np.float32)

    in_maps = []
    bf = ml_dtypes.bfloat16
    for core in range(NCORES):
        b, br = divmod(core, 2)
        x1f = np.ascontiguousarray(x1[b].reshape(C, N).reshape(2, 128, N))
        x2f = np.ascontiguousarray(x2[b].reshape(C, N).reshape(2, 128, N))
        in_maps.append({
            "xqk": (x1f if br == 0 else x2f).astype(np.float16),
            "xv": (x2f if br == 0 else x1f).astype(bf),
            "xcr": x1f.astype(bf),
            "wqk": wqk, "wvca": wvca, "bqe": bqe,
        })
    return in_maps


def assemble(results):
    """results: list of 8 dicts with 'out' [1, N] -> (out1, out2) full."""
    outs = []
    for br in range(2):
        full = np.empty((B, 1, HH, WW), np.float32)
        for b in range(B):
            full[b, 0] = results[2 * b + br]["out"][0].reshape(HH, WW)
        outs.append(full)
    return outs[0], outs[1]


def kernel(x1, x2, Wq, bq, Wk, bk, Wv, bv, Wc, bc):
    in_maps = host_inputs(x1, x2, Wq, bq, Wk, bk, Wv, bv, Wc, bc)
    nc = _get_nc()
    res = run_bass_kernel_spmd(nc, in_maps, core_ids=list(range(NCORES)))
    return assemble(res.results)


# revision 68
# speedup vs baseline: 1.0207x; 1.0038x over previous
"""Trainium2 Bass kernel for nn_CrossAttention (B=4, C=256, H=W=64).

Sharding: 8 cores = (batch b, branch br). Each core computes ONE branch's
full [1, N] output row for its batch:
  br=0: q,k from x1, v from x2;  br=1: q,k from x2, v from x1.
Host passes role-named inputs (xqk, xv, xcr=x1-for-combine) so the SPMD
program is branch-agnostic. This halves the k/v projection work vs
query-half sharding (no duplication across the batch pair).

Per core, for all N=4096 query rows i:
  q = Wq xqk + bq        [32, 4096] stored 4x row-replicated as q4 [128, N]
  k = Wk xqk             [32, 4096] 4x row-replicated   (bk softmax-invariant)
  vT = (Wv xv)^T         [4096, 256] bf16  (bv folded into bc_eff on host)
  S^T[j, i] = k_j . q_i  ; E = exp(S^T) bf16  (|S| <~ 30, exp safe in f32)
  r[i] = sum_j E[j, i]   via DVE bf16 accumulation (2 interleaved accums)
                         + one K=128 ones-matmul fold per block
  1/r via reciprocal_approx_fast; broadcast to 128 partitions via a K=1
  ones outer-product matmul (no DRAM round trip)
  att = (vT^T E) / r ; comb = Wc [xcr; att] + bc_eff ; out = sum_c |comb|

Pipelining: 8 blocks of 512 query cols; within a block the attended
matmuls run one j-group behind the score matmuls (PE never waits on Act
exp); each block's tail (r fold/recip/broadcast/normalize/combine) is
issued interleaved into the NEXT block's matmul stream, so the PE queue
never drains and HAM stays at 2.4 GHz. Scores issue as QUADS of 4
matmuls on disjoint 32-row PE strips (they stream concurrently: the
2nd-4th cost ~10ns each). The xcr combine input is DMA'd late on the
scalar queue, hidden under the first blocks' compute.
PSUM: 4 score staging + 3 attended accumulators + 1 rotating tail bank.
"""

import numpy as np
import ml_dtypes

import concourse.bass as bass
import concourse.bacc as bacc
import concourse.tile as tile
import concourse.mybir as mybir
from concourse.bass_utils import run_bass_kernel_spmd

B, C, HH, WW = 4, 256, 64, 64
N = HH * WW          # 4096
CQK = 32
IH = N // 2
NCORES = 8
NJC = N // 128       # 32 key-dim 128-chunks
NG = NJC // 2        # 16 groups of 2 key-chunks
NBLK = N // 512      # 8 query blocks

F32 = mybir.dt.float32
F32R = mybir.dt.float32r
BF16 = mybir.dt.bfloat16
F16 = mybir.dt.float16
AF = mybir.ActivationFunctionType


def build_program(nc, tc):
    # ---- DRAM I/O ------------------------------------------------------
    dram = {}
    # xqk/wq/wk in fp16: x quantization at 0.05% keeps the score logits
    # accurate (bf16's 0.4% would shift competing logits too much), the
    # projections accumulate in f32, and fp16 matmuls run at full rate.
    for name, shape, dt in [
        ("xqk", [2, 128, N], F16), ("xv", [2, 128, N], BF16),
        ("xcr", [2, 128, N], BF16),
        ("wqk", [128, 2, 256], F16),       # wq || wk along last axis
        ("wvca", [128, 2, 3 * C], BF16),   # wv || wcx || wca
        ("bqe", [128, 3], F32),            # bq || bce
    ]:
        dram[name] = nc.dram_tensor(name, shape, dt, kind="ExternalInput").ap()
    out_d = nc.dram_tensor("out", [1, N], F32, kind="ExternalOutput").ap()

    import contextlib
    with contextlib.ExitStack() as ctx:
        persist = ctx.enter_context(tc.tile_pool(name="persist", bufs=1))

        wqk_sb = persist.tile([128, 2, 256], F16, tag="wqk")
        wvca_sb = persist.tile([128, 2, 3 * C], BF16, tag="wvca")
        bqe_sb = persist.tile([128, 3], F32, tag="bqe")
        ones_bf = persist.tile([128, 1], BF16, tag="ones")
        ones_row = persist.tile([1, 128], BF16, tag="ones_row")

        # weights packed host-side into 3 tensors = 3 dma_starts (each
        # start costs ~630ns of sequencer issue time), issued from the
        # (otherwise idle) scalar queue so they don't delay the x loads.
        nc.scalar.dma_start(out=wqk_sb, in_=dram["wqk"])
        nc.scalar.dma_start(out=wvca_sb, in_=dram["wvca"])
        nc.scalar.dma_start(out=bqe_sb, in_=dram["bqe"])
        nc.vector.memset(ones_bf, 1.0)
        nc.vector.memset(ones_row, 1.0)

        def wq_s(kc):
            return wqk_sb[:, kc, 0:128]

        def wk_s(kc):
            return wqk_sb[:, kc, 128:256]

        def wv_s(kc):
            return wvca_sb[:, kc, 0:C]

        def wcx_s(kc, c2):
            return wvca_sb[:, kc, bass.ds(C + c2 * 128, 128)]

        def wca_s(kc, c2):
            return wvca_sb[:, kc, bass.ds(2 * C + c2 * 128, 128)]

        bq_sb = bqe_sb[:, 0:1]
        bce_sb = bqe_sb[:, 1:3]

        q4_sb = persist.tile([128, N], F32R, tag="q4")
        k4_sb = [persist.tile([128, IH], F32R, tag=f"k{h}", name=f"k{h}")
                 for h in range(2)]
        vT_sb = [persist.tile([128, (NJC // 2) * C], BF16, tag=f"vt{h}",
                              name=f"vt{h}") for h in range(2)]
        att_sb = [persist.tile([128, N], BF16, tag=f"att{c2}",
                               name=f"att{c2}") for c2 in range(2)]
        xcr_sb = persist.tile([128, 2, N], BF16, tag="xcr")

        # ---- phase 1: projections -------------------------------------
        with tc.tile_pool(name="proj_sb", bufs=4) as proj_sb, \
             tc.tile_pool(name="ps_kq", bufs=3, space="PSUM") as ps_kq, \
             tc.tile_pool(name="ps_vt", bufs=2, space="PSUM") as ps_vt:

            xq = [proj_sb.tile([128, 2, IH], F16, tag="xq", name=f"xq{h}")
                  for h in range(2)]
            xvt = [proj_sb.tile([128, 2, IH], BF16, tag="xv", name=f"xv{h}")
                   for h in range(2)]

            def load_half(src, dst, h, eng, nchunk):
                # chunks in consumption order: dma_start issue costs
                # ~630ns (sync) / ~1us (gpsimd SWDGE) of sequencer time,
                # so chunk count balances issue rate against letting the
                # first projection start early. xqk via sync, xv via the
                # idle gpsimd queue so both streams issue in parallel.
                cw = IH // nchunk
                for jb in range(nchunk):
                    sl = bass.ds(jb * cw, cw)
                    for kc in range(2):
                        eng.dma_start(
                            out=dst[:, kc, sl],
                            in_=dram[src][kc][:, h * IH + jb * cw:
                                              h * IH + (jb + 1) * cw])

            load_half("xqk", xq[0], 0, nc.sync, 4)
            load_half("xv", xvt[0], 0, nc.gpsimd, 2)
            load_half("xqk", xq[1], 1, nc.sync, 2)
            load_half("xv", xvt[1], 1, nc.scalar, 2)

            def qk_proj(h):
                xap = [xq[h][:, 0, :], xq[h][:, 1, :]]
                for jb in range(4):
                    sl = bass.ts(jb, 512)
                    osl = bass.ts(h * 4 + jb, 512)
                    qp = ps_kq.tile([128, 512], F32, tag="kq", name="qp")
                    for kc in range(2):
                        nc.tensor.matmul(qp, wq_s(kc), xap[kc][:, sl],
                                         start=(kc == 0), stop=(kc == 1))
                    nc.scalar.activation(q4_sb[:, osl], qp, AF.Identity,
                                         bias=bq_sb)
                    kp = ps_kq.tile([128, 512], F32, tag="kq", name="kp")
                    for kc in range(2):
                        nc.tensor.matmul(kp, wk_s(kc), xap[kc][:, sl],
                                         start=(kc == 0), stop=(kc == 1))
                    # k drain on DVE: keeps Act free for the first block's
                    # exps (Act otherwise enters phase 2 with a backlog)
                    nc.vector.tensor_copy(k4_sb[h][:, sl], kp)

            def v_proj(h):
                xap = [xvt[h][:, 0, :], xvt[h][:, 1, :]]
                for g in range(4):
                    vtp = ps_vt.tile([128, 4, C], F32, tag="vt", name="vtp")
                    for s in range(4):
                        jsub = g * 4 + s
                        for kc in range(2):
                            nc.tensor.matmul(
                                vtp[:, s, :],
                                xap[kc][:, bass.ts(jsub, 128)],
                                wv_s(kc),
                                start=(kc == 0), stop=(kc == 1))
                    nc.vector.tensor_copy(
                        vT_sb[h][:, bass.ds(g * 4 * C, 4 * C)],
                        vtp.rearrange("p a c -> p (a c)"))

            qk_proj(0)
            qk_proj(1)
            # combine input load: issued on the scalar queue HERE so its
            # transfer starts only after the projection inputs are on the
            # wire (needed first by block 0's tail, ~20us into phase 2).
            for kc in range(2):
                nc.scalar.dma_start(out=xcr_sb[:, kc, :], in_=dram["xcr"][kc])
            v_proj(0)
            v_proj(1)

        # ---- phase 2: attention + fused combine, 1-block pipelined ----
        with tc.tile_pool(name="attn_sb", bufs=1) as attn_sb, \
             tc.tile_pool(name="ps_st", bufs=1, space="PSUM") as ps_st, \
             tc.tile_pool(name="ps_att", bufs=1, space="PSUM") as ps_att, \
             tc.tile_pool(name="ps_tail", bufs=1, space="PSUM") as ps_tail:

            state = {}

            def tail_ops(p, g, pst, epi=False):
                pisl = bass.ts(p, 512)
                if g == 0:
                    for c2 in range(2):
                        nc.vector.tensor_copy(att_sb[c2][:, pisl],
                                              pst["attp"][c2])
                elif g == 1:
                    rft = ps_tail.tile([1, 512], F32, tag="tail", name="rft")
                    nc.tensor.matmul(rft, ones_bf, pst["racc"][0],
                                     start=True, stop=False)
                    nc.tensor.matmul(rft, ones_bf, pst["racc"][1],
                                     start=False, stop=True)
                    pst["rft"] = rft
                elif g == 2:
                    rr = attn_sb.tile([1, 512], F32, tag="rr", bufs=2,
                                      name="rr")
                    nc.vector.reciprocal_approx_fast(out=rr, in_=pst["rft"])
                    rr_bf = attn_sb.tile([1, 512], BF16, tag="rr_bf", bufs=2,
                                         name="rr_bf")
                    nc.vector.tensor_copy(rr_bf, rr)
                    pst["rr_bf"] = rr_bf
                elif g == 5:
                    rrb = ps_tail.tile([128, 512], F32, tag="tail",
                                       name="rrb")
                    nc.tensor.matmul(rrb, ones_row, pst["rr_bf"],
                                     start=True, stop=True)
                    pst["rrb"] = rrb
                elif g == 6:
                    for c2 in range(2):
                        a = att_sb[c2][:, pisl]
                        nc.vector.tensor_mul(a, a, pst["rrb"])
                elif g == 8 or g == 10:
                    c2 = 0 if g == 8 else 1
                    if epi:
                        # final block: scores are done, so the stp banks
                        # are free — dodging the tail-bank serialization
                        # shortens the end-of-kernel drain chain.
                        cp = ps_st.tile([128, 2, 512], F32, tag="stp",
                                        bufs=2, name=f"cpe{c2}")[:, 0, :]
                    else:
                        cp = ps_tail.tile([128, 512], F32, tag="tail",
                                          name=f"cp{c2}")
                    for kc in range(2):
                        nc.tensor.matmul(cp, wcx_s(kc, c2),
                                         xcr_sb[:, kc, pisl],
                                         start=(kc == 0), stop=False)
                    for kc in range(2):
                        nc.tensor.matmul(cp, wca_s(kc, c2),
                                         att_sb[kc][:, pisl],
                                         start=False, stop=(kc == 1))
                    absb = attn_sb.tile([128, 512], BF16, tag="absb",
                                        bufs=4, name=f"absb{c2}")
                    nc.scalar.activation(absb, cp, AF.Abs,
                                         bias=bce_sb[:, c2:c2 + 1])
                    pst[f"absb{c2}"] = absb
                elif g == 12:
                    outp = ps_tail.tile([1, 512], F32, tag="tail",
                                        name="outp")
                    nc.tensor.matmul(outp, ones_bf, pst["absb0"],
                                     start=True, stop=False)
                    nc.tensor.matmul(outp, ones_bf, pst["absb1"],
                                     start=False, stop=True)
                    pst["outp"] = outp
                elif g == 13:
                    osb = attn_sb.tile([1, 512], F32, tag="osb", bufs=2,
                                       name="osb")
                    nc.vector.tensor_copy(osb, pst["outp"])
                    nc.sync.dma_start(out=out_d[0:1, pisl], in_=osb)

            def attended(st, g, first, last):
                jh = g // (NG // 2)
                for t in range(2):
                    jloc = (2 * g + t) - jh * (NJC // 2)
                    for c2 in range(2):
                        nc.tensor.matmul(
                            st["attp"][c2],
                            vT_sb[jh][:, bass.ds(jloc * C + c2 * 128, 128)],
                            st["est"][g % 8][:, t, :],
                            start=(first and t == 0),
                            stop=(last and t == 1))

            for mi in range(NBLK + 1):
                cur = mi if mi < NBLK else None
                if cur is not None:
                    isl = bass.ts(cur, 512)
                    st = {
                        "attp": [ps_att.tile([128, 512], F32, tag="attp",
                                             bufs=3, name=f"attp{c2}")
                                 for c2 in range(2)],
                        "racc": [attn_sb.tile([128, 512], BF16, tag="racc",
                                              bufs=4, name=f"racc{t}")
                                 for t in range(2)],
                        "est": [None] * 8,
                    }
                    state[mi] = st
                for g in range(16 if cur is not None else 14):
                    if cur is not None:
                        if g % 2 == 0:
                            # score QUAD: 4 chunks (2 groups) issued as
                            # back-to-back matmuls on disjoint 32-row PE
                            # strips — they stream concurrently, so the
                            # quad costs ~1 matmul, not 4. The 4x q/k row
                            # replication provides all 4 strips.
                            stps = [ps_st.tile([128, 2, 512], F32,
                                               tag="stp", bufs=2,
                                               name=f"stp{u}")
                                    for u in range(2)]
                            st["stp"] = stps
                            jh = g // (NG // 2)
                            for u in range(4):
                                c = 4 * (g // 2) + u
                                jloc = c - jh * (NJC // 2)
                                nc.tensor.matmul(
                                    stps[u // 2][:, u % 2, :],
                                    k4_sb[jh][bass.ds(32 * u, 32),
                                              bass.ts(jloc, 128)],
                                    q4_sb[bass.ds(32 * u, 32), isl],
                                    start=True, stop=True,
                                    tile_position=(32 * u, 0))
                        stp = st["stp"][g % 2]
                        est = attn_sb.tile([128, 2, 512], BF16,
                                           tag="est", bufs=8, name="est")
                        st["est"][g % 8] = est
                        nc.scalar.activation(
                            est.rearrange("p a n -> p (a n)"),
                            stp.rearrange("p a n -> p (a n)"), AF.Exp)
                        # attended runs TWO groups behind the scores so
                        # the sem->exp->sem latency chain never paces the
                        # PE stream.
                        if g > 1:
                            attended(st, g - 2, first=(g == 2), last=False)
                        # DVE r-accumulation (bf16, 2 interleaved accums)
                        for t in range(2):
                            if g == 0:
                                nc.vector.tensor_copy(st["racc"][t],
                                                      est[:, t, :])
                            else:
                                nc.vector.tensor_add(st["racc"][t],
                                                     st["racc"][t],
                                                     est[:, t, :])
                    if mi > 0:
                        tail_ops(mi - 1, g, state[mi - 1], epi=(cur is None))
                if cur is not None:
                    attended(st, 14, first=False, last=False)
                    attended(st, 15, first=False, last=True)


_NC_CACHE = {}


def _get_nc():
    if "nc" not in _NC_CACHE:
        nc = bacc.Bacc("TRN2", debug=False, enable_asserts=False,
                       target_bir_lowering=False, enable_partition_id=False)
        with tile.TileContext(nc) as tc:
            build_program(nc, tc)
        nc.compile()
        _NC_CACHE["nc"] = nc
    return _NC_CACHE["nc"]


def host_inputs(x1, x2, Wq, bq, Wk, bk, Wv, bv, Wc, bc):
    """Build the 8 per-core input maps (host-side sharding/layout only)."""
    f = np.float32
    x1 = np.asarray(x1, f); x2 = np.asarray(x2, f)
    Wq = np.asarray(Wq, f); bq = np.asarray(bq, f)
    Wk = np.asarray(Wk, f)
    Wv = np.asarray(Wv, f); bv = np.asarray(bv, f)
    Wc = np.asarray(Wc, f); bc = np.asarray(bc, f)

    # 4x row-replicated q/k projection weights, packed [128, kc, out]
    Wq4 = np.tile(Wq, (4, 1))            # [128, 256]
    Wk4 = np.tile(Wk, (4, 1))
    wq_t = Wq4.T.reshape(2, 128, 128).transpose(1, 0, 2)
    wk_t = Wk4.T.reshape(2, 128, 128).transpose(1, 0, 2)
    wqk = np.ascontiguousarray(
        np.concatenate([wq_t, wk_t], axis=2)).astype(np.float16)
    wv_t = Wv.T.reshape(2, 128, C).transpose(1, 0, 2)
    WcT = np.ascontiguousarray(Wc.T)     # [512, 256]
    wcx_t = WcT[:C].reshape(2, 128, C).transpose(1, 0, 2)
    wca_t = WcT[C:].reshape(2, 128, C).transpose(1, 0, 2)
    wvca = np.ascontiguousarray(np.concatenate(
        [wv_t, wcx_t, wca_t], axis=2)).astype(ml_dtypes.bfloat16)
    bq4 = np.tile(bq, 4)                                    # [128]
    bce = (bc + Wc[:, C:] @ bv).reshape(2, 128).T           # [128, 2]
    bqe = np.ascontiguousarray(
        np.concatenate([bq4[:, None], bce], axis=1), np.float32)

    in_maps = []
    bf = ml_dtypes.bfloat16
    for core in range(NCORES):
        b, br = divmod(core, 2)
        x1f = np.ascontiguousarray(x1[b].reshape(C, N).reshape(2, 128, N))
        x2f = np.ascontiguousarray(x2[b].reshape(C, N).reshape(2, 128, N))
        in_maps.append({
            "xqk": (x1f if br == 0 else x2f).astype(np.float16),
            "xv": (x2f if br == 0 else x1f).astype(bf),
            "xcr": x1f.astype(bf),
            "wqk": wqk, "wvca": wvca, "bqe": bqe,
        })
    return in_maps


def assemble(results):
    """results: list of 8 dicts with 'out' [1, N] -> (out1, out2) full."""
    outs = []
    for br in range(2):
        full = np.empty((B, 1, HH, WW), np.float32)
        for b in range(B):
            full[b, 0] = results[2 * b + br]["out"][0].reshape(HH, WW)
        outs.append(full)
    return outs[0], outs[1]


def kernel(x1, x2, Wq, bq, Wk, bk, Wv, bv, Wc, bc):
    in_maps = host_inputs(x1, x2, Wq, bq, Wk, bk, Wv, bv, Wc, bc)
    nc = _get_nc()
    res = run_bass_kernel_spmd(nc, in_maps, core_ids=list(range(NCORES)))
    return assemble(res.results)


# revision 72
# speedup vs baseline: 1.0467x; 1.0254x over previous
"""Trainium2 Bass kernel for nn_CrossAttention (B=4, C=256, H=W=64).

Sharding: 8 cores = (batch b, branch br). Each core computes ONE branch's
full [1, N] output row for its batch:
  br=0: q,k from x1, v from x2;  br=1: q,k from x2, v from x1.
Host passes role-named inputs (xqk, xv, xcr=x1-for-combine) so the SPMD
program is branch-agnostic. This halves the k/v projection work vs
query-half sharding (no duplication across the batch pair).

Per core, for all N=4096 query rows i:
  q = Wq xqk + bq        [32, 4096] stored 4x row-replicated as q4 [128, N]
  k = Wk xqk             [32, 4096] 4x row-replicated   (bk softmax-invariant)
  vT = (Wv xv)^T         [4096, 256] bf16  (bv folded into bc_eff on host)
  S^T[j, i] = k_j . q_i  ; E = exp(S^T) bf16  (|S| <~ 30, exp safe in f32)
  r[i] = sum_j E[j, i]   via DVE bf16 accumulation (2 interleaved accums)
                         + one K=128 ones-matmul fold per block
  1/r via reciprocal_approx_fast; broadcast to 128 partitions via a K=1
  ones outer-product matmul (no DRAM round trip)
  att = (vT^T E) / r ; comb = Wc [xcr; att] + bc_eff ; out = sum_c |comb|

Pipelining: 8 blocks of 512 query cols; within a block the attended
matmuls run one j-group behind the score matmuls (PE never waits on Act
exp); each block's tail (r fold/recip/broadcast/normalize/combine) is
issued interleaved into the NEXT block's matmul stream, so the PE queue
never drains and HAM stays at 2.4 GHz. Scores issue as QUADS of 4
matmuls on disjoint 32-row PE strips (they stream concurrently: the
2nd-4th cost ~10ns each). The xcr combine input is DMA'd late on the
scalar queue, hidden under the first blocks' compute.
PSUM: 4 score staging + 3 attended accumulators + 1 rotating tail bank.
"""

import numpy as np
import ml_dtypes

import concourse.bass as bass
import concourse.bacc as bacc
import concourse.tile as tile
import concourse.mybir as mybir
from concourse.bass_utils import run_bass_kernel_spmd

B, C, HH, WW = 4, 256, 64, 64
N = HH * WW          # 4096
CQK = 32
IH = N // 2
NCORES = 8
NJC = N // 128       # 32 key-dim 128-chunks
NG = NJC // 2        # 16 groups of 2 key-chunks
NBLK = N // 512      # 8 query blocks

F32 = mybir.dt.float32
F32R = mybir.dt.float32r
BF16 = mybir.dt.bfloat16
F16 = mybir.dt.float16
AF = mybir.ActivationFunctionType


def build_program(nc, tc):
    # ---- DRAM I/O ------------------------------------------------------
    dram = {}
    # xqk/wq/wk in fp16: x quantization at 0.05% keeps the score logits
    # accurate (bf16's 0.4% would shift competing logits too much), the
    # projections accumulate in f32, and fp16 matmuls run at full rate.
    for name, shape, dt in [
        ("xqk", [2, 128, N], F16), ("xv", [2, 128, N], BF16),
        ("xcr", [2, 128, N], BF16),
        ("wqk", [128, 2, 256], F16),       # wq || wk along last axis
        ("wvca", [128, 2, 3 * C], BF16),   # wv || wcx || wca
        ("bqe", [128, 3], F32),            # bq || bce
    ]:
        dram[name] = nc.dram_tensor(name, shape, dt, kind="ExternalInput").ap()
    out_d = nc.dram_tensor("out", [1, N], F32, kind="ExternalOutput").ap()

    import contextlib
    with contextlib.ExitStack() as ctx:
        persist = ctx.enter_context(tc.tile_pool(name="persist", bufs=1))

        wqk_sb = persist.tile([128, 2, 256], F16, tag="wqk")
        wvca_sb = persist.tile([128, 2, 3 * C], BF16, tag="wvca")
        bqe_sb = persist.tile([128, 3], F32, tag="bqe")
        ones_bf = persist.tile([128, 1], BF16, tag="ones")
        ones_row = persist.tile([1, 128], BF16, tag="ones_row")

        # weights packed host-side into 3 tensors = 3 dma_starts (each
        # start costs ~630ns of sequencer issue time), issued from the
        # (otherwise idle) scalar queue so they don't delay the x loads.
        nc.scalar.dma_start(out=wqk_sb, in_=dram["wqk"])
        nc.scalar.dma_start(out=wvca_sb, in_=dram["wvca"])
        nc.scalar.dma_start(out=bqe_sb, in_=dram["bqe"])
        nc.vector.memset(ones_bf, 1.0)
        nc.vector.memset(ones_row, 1.0)

        def wq_s(kc):
            return wqk_sb[:, kc, 0:128]

        def wk_s(kc):
            return wqk_sb[:, kc, 128:256]

        def wv_s(kc):
            return wvca_sb[:, kc, 0:C]

        def wcx_s(kc, c2):
            return wvca_sb[:, kc, bass.ds(C + c2 * 128, 128)]

        def wca_s(kc, c2):
            return wvca_sb[:, kc, bass.ds(2 * C + c2 * 128, 128)]

        bq_sb = bqe_sb[:, 0:1]
        bce_sb = bqe_sb[:, 1:3]

        q4_sb = persist.tile([128, N], F32R, tag="q4")
        k4_sb = [persist.tile([128, IH], F32R, tag=f"k{h}", name=f"k{h}")
                 for h in range(2)]
        vT_sb = [persist.tile([128, (NJC // 2) * C], BF16, tag=f"vt{h}",
                              name=f"vt{h}") for h in range(2)]
        att_sb = [persist.tile([128, N], BF16, tag=f"att{c2}",
                               name=f"att{c2}") for c2 in range(2)]
        xcr_sb = persist.tile([128, 2, N], BF16, tag="xcr")

        # ---- phase 1: projections -------------------------------------
        # proj_sb stays open into phase 2: v_proj(1) is deferred into
        # block 0's g-loop (its matmuls use the then-idle tail PSUM bank)
        proj_sb = ctx.enter_context(tc.tile_pool(name="proj_sb", bufs=4))
        with tc.tile_pool(name="ps_kq", bufs=3, space="PSUM") as ps_kq, \
             tc.tile_pool(name="ps_vt", bufs=2, space="PSUM") as ps_vt:

            xq = [proj_sb.tile([128, 2, IH], F16, tag="xq", name=f"xq{h}")
                  for h in range(2)]
            xvt = [proj_sb.tile([128, 2, IH], BF16, tag="xv", name=f"xv{h}")
                   for h in range(2)]

            def load_half(src, dst, h, eng, nchunk):
                # chunks in consumption order: dma_start issue costs
                # ~630ns (sync) / ~1us (gpsimd SWDGE) of sequencer time,
                # so chunk count balances issue rate against letting the
                # first projection start early. xqk via sync, xv via the
                # idle gpsimd queue so both streams issue in parallel.
                cw = IH // nchunk
                for jb in range(nchunk):
                    sl = bass.ds(jb * cw, cw)
                    for kc in range(2):
                        eng.dma_start(
                            out=dst[:, kc, sl],
                            in_=dram[src][kc][:, h * IH + jb * cw:
                                              h * IH + (jb + 1) * cw])

            load_half("xqk", xq[0], 0, nc.sync, 4)
            load_half("xv", xvt[0], 0, nc.gpsimd, 2)
            load_half("xqk", xq[1], 1, nc.sync, 2)
            load_half("xv", xvt[1], 1, nc.scalar, 2)

            def qk_proj(h):
                xap = [xq[h][:, 0, :], xq[h][:, 1, :]]
                for jb in range(4):
                    sl = bass.ts(jb, 512)
                    osl = bass.ts(h * 4 + jb, 512)
                    qp = ps_kq.tile([128, 512], F32, tag="kq", name="qp")
                    for kc in range(2):
                        nc.tensor.matmul(qp, wq_s(kc), xap[kc][:, sl],
                                         start=(kc == 0), stop=(kc == 1))
                    nc.scalar.activation(q4_sb[:, osl], qp, AF.Identity,
                                         bias=bq_sb)
                    kp = ps_kq.tile([128, 512], F32, tag="kq", name="kp")
                    for kc in range(2):
                        nc.tensor.matmul(kp, wk_s(kc), xap[kc][:, sl],
                                         start=(kc == 0), stop=(kc == 1))
                    # k drain on DVE: keeps Act free for the first block's
                    # exps (Act otherwise enters phase 2 with a backlog)
                    nc.vector.tensor_copy(k4_sb[h][:, sl], kp)

            def v_proj(h):
                xap = [xvt[h][:, 0, :], xvt[h][:, 1, :]]
                for g in range(4):
                    vtp = ps_vt.tile([128, 4, C], F32, tag="vt", name="vtp")
                    for s in range(4):
                        jsub = g * 4 + s
                        for kc in range(2):
                            nc.tensor.matmul(
                                vtp[:, s, :],
                                xap[kc][:, bass.ts(jsub, 128)],
                                wv_s(kc),
                                start=(kc == 0), stop=(kc == 1))
                    nc.vector.tensor_copy(
                        vT_sb[h][:, bass.ds(g * 4 * C, 4 * C)],
                        vtp.rearrange("p a c -> p (a c)"))

            qk_proj(0)
            qk_proj(1)
            # combine input load: issued on the scalar queue HERE so its
            # transfer starts only after the projection inputs are on the
            # wire (needed first by block 0's tail, ~20us into phase 2).
            for kc in range(2):
                nc.scalar.dma_start(out=xcr_sb[:, kc, :], in_=dram["xcr"][kc])
            v_proj(0)
            # v_proj(1) deferred into block 0 (phase 2)

        # ---- phase 2: attention + fused combine, 1-block pipelined ----
        with tc.tile_pool(name="attn_sb", bufs=1) as attn_sb, \
             tc.tile_pool(name="ps_st", bufs=1, space="PSUM") as ps_st, \
             tc.tile_pool(name="ps_att", bufs=1, space="PSUM") as ps_att, \
             tc.tile_pool(name="ps_tail", bufs=1, space="PSUM") as ps_tail:

            state = {}

            def tail_ops(p, g, pst, epi=False):
                pisl = bass.ts(p, 512)
                if g == 0:
                    for c2 in range(2):
                        nc.vector.tensor_copy(att_sb[c2][:, pisl],
                                              pst["attp"][c2])
                elif g == 1:
                    rft = ps_tail.tile([1, 512], F32, tag="tail", name="rft")
                    nc.tensor.matmul(rft, ones_bf, pst["racc"][0],
                                     start=True, stop=False)
                    nc.tensor.matmul(rft, ones_bf, pst["racc"][1],
                                     start=False, stop=True)
                    pst["rft"] = rft
                elif g == 2:
                    rr = attn_sb.tile([1, 512], F32, tag="rr", bufs=2,
                                      name="rr")
                    nc.vector.reciprocal_approx_fast(out=rr, in_=pst["rft"])
                    rr_bf = attn_sb.tile([1, 512], BF16, tag="rr_bf", bufs=2,
                                         name="rr_bf")
                    nc.vector.tensor_copy(rr_bf, rr)
                    pst["rr_bf"] = rr_bf
                elif g == 5:
                    rrb = ps_tail.tile([128, 512], F32, tag="tail",
                                       name="rrb")
                    nc.tensor.matmul(rrb, ones_row, pst["rr_bf"],
                                     start=True, stop=True)
                    pst["rrb"] = rrb
                elif g == 6:
                    for c2 in range(2):
                        a = att_sb[c2][:, pisl]
                        nc.vector.tensor_mul(a, a, pst["rrb"])
                elif g == 8 or g == 10:
                    c2 = 0 if g == 8 else 1
                    if epi:
                        # final block: scores are done, so the stp banks
                        # are free — dodging the tail-bank serialization
                        # shortens the end-of-kernel drain chain.
                        cp = ps_st.tile([128, 2, 512], F32, tag="stp",
                                        bufs=2, name=f"cpe{c2}")[:, 0, :]
                    else:
                        cp = ps_tail.tile([128, 512], F32, tag="tail",
                                          name=f"cp{c2}")
                    for kc in range(2):
                        nc.tensor.matmul(cp, wcx_s(kc, c2),
                                         xcr_sb[:, kc, pisl],
                                         start=(kc == 0), stop=False)
                    for kc in range(2):
                        nc.tensor.matmul(cp, wca_s(kc, c2),
                                         att_sb[kc][:, pisl],
                                         start=False, stop=(kc == 1))
                    absb = attn_sb.tile([128, 512], BF16, tag="absb",
                                        bufs=4, name=f"absb{c2}")
                    nc.scalar.activation(absb, cp, AF.Abs,
                                         bias=bce_sb[:, c2:c2 + 1])
                    pst[f"absb{c2}"] = absb
                elif g == 12:
                    outp = ps_tail.tile([1, 512], F32, tag="tail",
                                        name="outp")
                    nc.tensor.matmul(outp, ones_bf, pst["absb0"],
                                     start=True, stop=False)
                    nc.tensor.matmul(outp, ones_bf, pst["absb1"],
                                     start=False, stop=True)
                    pst["outp"] = outp
                elif g == 13:
                    osb = attn_sb.tile([1, 512], F32, tag="osb", bufs=2,
                                       name="osb")
                    nc.vector.tensor_copy(osb, pst["outp"])
                    nc.sync.dma_start(out=out_d[0:1, pisl], in_=osb)

            def attended(st, g, first, last, ts=(0, 1)):
                jh = g // (NG // 2)
                for t in ts:
                    jloc = (2 * g + t) - jh * (NJC // 2)
                    for c2 in range(2):
                        nc.tensor.matmul(
                            st["attp"][c2],
                            vT_sb[jh][:, bass.ds(jloc * C + c2 * 128, 128)],
                            st["est"][g % 8][:, t, :],
                            start=(first and t == 0),
                            stop=(last and t == 1))

            def v1_rot(r):
                # deferred v_proj(1): one rotation of the (idle during
                # block 0) tail PSUM bank covers 2 j-subchunks
                vtp = ps_tail.tile([128, 2, C], F32, tag="tail", name="vtp")
                for i in range(2):
                    for kc in range(2):
                        nc.tensor.matmul(
                            vtp[:, i, :],
                            xvt[1][:, kc, bass.ts(2 * r + i, 128)],
                            wv_s(kc), start=(kc == 0), stop=(kc == 1))
                nc.vector.tensor_copy(
                    vT_sb[1][:, bass.ds(2 * r * C, 2 * C)],
                    vtp.rearrange("p a c -> p (a c)"))

            for mi in range(NBLK + 1):
                cur = mi if mi < NBLK else None
                if cur is not None:
                    isl = bass.ts(cur, 512)
                    st = {
                        "attp": [ps_att.tile([128, 512], F32, tag="attp",
                                             bufs=3, name=f"attp{c2}")
                                 for c2 in range(2)],
                        "racc": [attn_sb.tile([128, 512], BF16, tag="racc",
                                              bufs=4, name=f"racc{t}")
                                 for t in range(2)],
                        "est": [None] * 8,
                    }
                    state[mi] = st
                for g in range(16 if cur is not None else 14):
                    if cur is not None:
                        if g % 2 == 0:
                            # half the lagged attended BEFORE the quad:
                            # gives exp(g-1) extra time to free its stp
                            # buffer and hides the quad's weight loads
                            # under attended streaming.
                            if g > 1:
                                attended(st, g - 2, first=(g == 2),
                                         last=False, ts=(0,))
                            if mi == 0:
                                v1_rot(g // 2)
                            # score QUAD: 4 chunks (2 groups) issued as
                            # back-to-back matmuls on disjoint 32-row PE
                            # strips — they stream concurrently, so the
                            # quad costs ~1 matmul, not 4. The 4x q/k row
                            # replication provides all 4 strips.
                            stps = [ps_st.tile([128, 2, 512], F32,
                                               tag="stp", bufs=2,
                                               name=f"stp{u}")
                                    for u in range(2)]
                            st["stp"] = stps
                            jh = g // (NG // 2)
                            for u in range(4):
                                c = 4 * (g // 2) + u
                                jloc = c - jh * (NJC // 2)
                                nc.tensor.matmul(
                                    stps[u // 2][:, u % 2, :],
                                    k4_sb[jh][bass.ds(32 * u, 32),
                                              bass.ts(jloc, 128)],
                                    q4_sb[bass.ds(32 * u, 32), isl],
                                    start=True, stop=True,
                                    tile_position=(32 * u, 0))
                        stp = st["stp"][g % 2]
                        est = attn_sb.tile([128, 2, 512], BF16,
                                           tag="est", bufs=8, name="est")
                        st["est"][g % 8] = est
                        nc.scalar.activation(
                            est.rearrange("p a n -> p (a n)"),
                            stp.rearrange("p a n -> p (a n)"), AF.Exp)
                        # attended runs TWO groups behind the scores so
                        # the sem->exp->sem latency chain never paces the
                        # PE stream.
                        if g > 1:
                            attended(st, g - 2, first=False, last=False,
                                     ts=(1,) if g % 2 == 0 else (0, 1))
                        # DVE r-accumulation (bf16, 2 interleaved accums)
                        for t in range(2):
                            if g == 0:
                                nc.vector.tensor_copy(st["racc"][t],
                                                      est[:, t, :])
                            else:
                                nc.vector.tensor_add(st["racc"][t],
                                                     st["racc"][t],
                                                     est[:, t, :])
                    if mi > 0:
                        tail_ops(mi - 1, g, state[mi - 1], epi=(cur is None))
                if cur is not None:
                    attended(st, 14, first=False, last=False)
                    attended(st, 15, first=False, last=True)


_NC_CACHE = {}


def _get_nc():
    if "nc" not in _NC_CACHE:
        nc = bacc.Bacc("TRN2", debug=False, enable_asserts=False,
                       target_bir_lowering=False, enable_partition_id=False)
        with tile.TileContext(nc) as tc:
            build_program(nc, tc)
        nc.compile()
        _NC_CACHE["nc"] = nc
    return _NC_CACHE["nc"]


def host_inputs(x1, x2, Wq, bq, Wk, bk, Wv, bv, Wc, bc):
    """Build the 8 per-core input maps (host-side sharding/layout only)."""
    f = np.float32
    x1 = np.asarray(x1, f); x2 = np.asarray(x2, f)
    Wq = np.asarray(Wq, f); bq = np.asarray(bq, f)
    Wk = np.asarray(Wk, f)
    Wv = np.asarray(Wv, f); bv = np.asarray(bv, f)
    Wc = np.asarray(Wc, f); bc = np.asarray(bc, f)

    # 4x row-replicated q/k projection weights, packed [128, kc, out]
    Wq4 = np.tile(Wq, (4, 1))            # [128, 256]
    Wk4 = np.tile(Wk, (4, 1))
    wq_t = Wq4.T.reshape(2, 128, 128).transpose(1, 0, 2)
    wk_t = Wk4.T.reshape(2, 128, 128).transpose(1, 0, 2)
    wqk = np.ascontiguousarray(
        np.concatenate([wq_t, wk_t], axis=2)).astype(np.float16)
    wv_t = Wv.T.reshape(2, 128, C).transpose(1, 0, 2)
    WcT = np.ascontiguousarray(Wc.T)     # [512, 256]
    wcx_t = WcT[:C].reshape(2, 128, C).transpose(1, 0, 2)
    wca_t = WcT[C:].reshape(2, 128, C).transpose(1, 0, 2)
    wvca = np.ascontiguousarray(np.concatenate(
        [wv_t, wcx_t, wca_t], axis=2)).astype(ml_dtypes.bfloat16)
    bq4 = np.tile(bq, 4)                                    # [128]
    bce = (bc + Wc[:, C:] @ bv).reshape(2, 128).T           # [128, 2]
    bqe = np.ascontiguousarray(
        np.concatenate([bq4[:, None], bce], axis=1), np.float32)

    in_maps = []
    bf = ml_dtypes.bfloat16
    for core in range(NCORES):
        b, br = divmod(core, 2)
        x1f = np.ascontiguousarray(x1[b].reshape(C, N).reshape(2, 128, N))
        x2f = np.ascontiguousarray(x2[b].reshape(C, N).reshape(2, 128, N))
        in_maps.append({
            "xqk": (x1f if br == 0 else x2f).astype(np.float16),
            "xv": (x2f if br == 0 else x1f).astype(bf),
            "xcr": x1f.astype(bf),
            "wqk": wqk, "wvca": wvca, "bqe": bqe,
        })
    return in_maps


def assemble(results):
    """results: list of 8 dicts with 'out' [1, N] -> (out1, out2) full."""
    outs = []
    for br in range(2):
        full = np.empty((B, 1, HH, WW), np.float32)
        for b in range(B):
            full[b, 0] = results[2 * b + br]["out"][0].reshape(HH, WW)
        outs.append(full)
    return outs[0], outs[1]


def kernel(x1, x2, Wq, bq, Wk, bk, Wv, bv, Wc, bc):
    in_maps = host_inputs(x1, x2, Wq, bq, Wk, bk, Wv, bv, Wc, bc)
    nc = _get_nc()
    res = run_bass_kernel_spmd(nc, in_maps, core_ids=list(range(NCORES)))
    return assemble(res.results)
